# revision 1
# baseline (speedup 1.0000x reference)
"""Trainium2 (Bass/Tile) segment-sum kernel, 8-core SPMD.

Computes out[v, :] = sum over rows n with X_node[n] == v of H[n, :]
(equivalent to jax.ops.segment_sum(H, X_node, num_segments=V)).

Strategy:
  host: stable-argsort rows by segment id; split the sorted order into 8
    contiguous chunks (one per NeuronCore) so each core covers a narrow,
    contiguous segment range (~V/8 segments). Within a core, rows are
    greedily grouped into W windows, each covering <=128 consecutive
    segments and <=T*128 rows; each window is laid out as T tiles of 128
    rows, padded with dummy rows (lid=255) so all 8 cores run ONE static
    SPMD program. The DRAM layout is pre-swizzled so each window is a
    single descriptor-efficient DMA ([128 partitions] x contiguous runs).
  precision: H is split as H = hi + lo with hi = bf16(H) and
    lo = bf16(H - hi) (two bf16 planes = same DMA bytes as f32, ~2^-17
    relative representation error). The one-hot weights are exactly 0/1
    in bf16, and the PE accumulates in fp32 PSUM, so the result matches
    the f32 reference to ~1e-5 relative. bf16 matmuls run the PE at
    2.4 GHz with hidden weight loads (~59 ns per 128x128x128), vs fp32's
    ~224 ns -- this moves the kernel from PE-bound to DMA-bound.
  device, per (window w, tile t): VectorE builds the one-hot stationary
    matrix onehot[n, v] = (lid[n] == v) with one fused is_equal per
    window; TensorE accumulates PSUM[v, d] += onehot^T @ Hhi_tile
    + onehot^T @ Hlo_tile over the window's T tiles (a cross-partition
    segmented reduce); PSUM is copied to SBUF and DMA'd out per window.
  host: add the per-core [W, 128, D] window strips into the full [V, D]
    output (windows of adjacent cores may overlap; addition is exact).

Per tile, ONE wide matmul streams hi|lo as a 256-wide moving operand into
a [v, 2*128] PSUM accumulator (halves the PE instruction count); the two
per-plane partial sums are added during PSUM evacuation.

Measured on the target data: ~293 us HW exec across 8 cores (f32 DMA
roofline ~285 us; shared-machine noise can add up to ~40 us), relative
error 2.5e-6 vs the f32 reference. Setting SEGSUM_PLANES=1 ships H as a
single bf16 plane instead of hi+lo (~233 us, relative error ~1.7e-3).
"""

import os

import numpy as np
from contextlib import ExitStack

import ml_dtypes
import concourse.bass as bass
import concourse.tile as tile
from concourse import bacc, mybir
from concourse.bass_utils import run_bass_kernel_spmd

F32 = mybir.dt.float32
BF16 = mybir.dt.bfloat16
NP_BF16 = ml_dtypes.bfloat16
P = 128  # partitions / tile rows / max window width (segments)
D = 128  # feature dim
N_CORES = 8
T_CANDIDATES = (28, 29, 30, 31, 32)  # tiles (of 128 rows) per window
PAD_LID = 255.0

LAST_RESULTS = None  # test-harness hook: BassKernelResults of the last run
_NC_CACHE = {}  # (W, T, planes) -> compiled Bacc program


def _build_nc_cached(W: int, T: int, planes: int):
    key = (W, T, planes)
    if key not in _NC_CACHE:
        _NC_CACHE[key] = _build_nc(W, T, planes)
    return _NC_CACHE[key]


def _build_nc(W: int, T: int, planes: int):
    nc = bacc.Bacc(
        "TRN2",
        target_bir_lowering=False,
        debug=False,
        enable_asserts=False,
        num_devices=N_CORES,
    )
    # h[w, p, (t, {hi,lo}, d)] -- per-partition contiguous runs of T*2*D*2B
    h = nc.dram_tensor("h", [W, P, T * planes * D], BF16, kind="ExternalInput")
    lid = nc.dram_tensor("lid", [P, W * T], BF16, kind="ExternalInput")
    iota = nc.dram_tensor("iota", [P, P], BF16, kind="ExternalInput")
    out = nc.dram_tensor("out", [W, P, D], F32, kind="ExternalOutput")

    with tile.TileContext(nc) as tc, ExitStack() as ctx:
        const = ctx.enter_context(tc.tile_pool(name="const", bufs=1))
        hpool = ctx.enter_context(tc.tile_pool(name="hw", bufs=8))
        ohpool = ctx.enter_context(tc.tile_pool(name="oh", bufs=4))
        opool = ctx.enter_context(tc.tile_pool(name="ot", bufs=8))
        psum = ctx.enter_context(tc.tile_pool(name="acc", bufs=4, space="PSUM"))

        halves = [(0, T // 2), (T // 2, T)]

        # issue the first windows' loads before the constants so the SDMA
        # engines have bulk work immediately
        def load_h(w, t0, t1):
            ht = hpool.tile([P, (t1 - t0) * planes * D], BF16, tag="ht")
            nc.sync.dma_start(ht[:], h[w][:, t0 * planes * D : t1 * planes * D])
            return ht

        hts = {}
        for w in range(2):
            for t0, t1 in halves:
                hts[(w, t0)] = load_h(w, t0, t1)

        # constants go via the ACT ring so they land immediately instead of
        # queueing behind the hoisted bulk loads on the SP ring
        iota_sb = const.tile([P, P], BF16)
        nc.scalar.dma_start(iota_sb[:], iota[:])
        lid_sb = const.tile([P, W * T], BF16)
        nc.scalar.dma_start(lid_sb[:], lid[:])

        for w in range(W):
            # one wide matmul per tile streams all planes; the psum holds
            # per-plane partial sums side by side, added at window end
            acc = psum.tile([P, planes * D], F32)
            for t0, t1 in halves:
                th = t1 - t0
                if (w, t0) in hts:
                    ht = hts[(w, t0)]
                else:
                    ht = load_h(w, t0, t1)
                # one fused DVE op builds this half-window's one-hot tiles:
                # oh[p, t, v] = (iota[p, v] == lid[p, w*T + t0 + t])
                oh = ohpool.tile([P, th, P], BF16)
                nc.vector.tensor_tensor(
                    oh[:],
                    iota_sb[:].unsqueeze(1).broadcast_to((P, th, P)),
                    lid_sb[:, w * T + t0 : w * T + t1]
                    .unsqueeze(2)
                    .broadcast_to((P, th, P)),
                    mybir.AluOpType.is_equal,
                )
                for t in range(th):
                    nc.tensor.matmul(
                        acc[:],
                        oh[:, t, :],
                        ht[:, planes * t * D : planes * (t + 1) * D],
                        start=(t0 == 0 and t == 0),
                        stop=(t1 == T and t == th - 1),
                    )
            ot = opool.tile([P, D], F32)
            nc.scalar.copy(ot[:], acc[:, :D])
            if planes == 2:
                # DVE allows only one PSUM operand per op
                nc.vector.tensor_tensor(
                    ot[:], ot[:], acc[:, D:], mybir.AluOpType.add
                )
            nc.scalar.dma_start(out[w], ot[:])

    nc.compile()
    return nc


def _prepare(H: np.ndarray, X: np.ndarray, V: int, planes: int):
    """Host-side sort + greedy windowing + hi/lo split + swizzle.

    Returns (in_maps, wbase[k, w] window base segments, W, T).
    """
    N, Dd = H.shape
    assert Dd == D and N % N_CORES == 0
    nloc = N // N_CORES
    X = np.ascontiguousarray(X).astype(np.int64, copy=False)
    perm = np.argsort(X, kind="stable")
    sidx = X[perm]

    def greedy(T):
        # greedy windows per core: <=T*128 rows and <=128-segment span each
        cap = T * P
        bounds = []  # per core: row-rank boundaries [0, ..., nloc]
        for k in range(N_CORES):
            s = sidx[k * nloc : (k + 1) * nloc]
            b = [0]
            r = 0
            while r < nloc:
                r = min(r + cap, int(np.searchsorted(s, s[r] + P, side="left")))
                b.append(r)
            bounds.append(np.asarray(b, np.int64))
        return bounds, max(len(b) - 1 for b in bounds)

    best = None
    for T in T_CANDIDATES:
        bounds, W = greedy(T)
        if best is None or W * T < best[2] * best[1]:
            best = (bounds, T, W)
    bounds, T, W = best
    cap = T * P

    # per-row window index / rank / local segment id
    wbase = np.full((N_CORES, W), V, np.int64)  # pad windows point past V
    win = np.empty(N, np.int64)
    rank = np.empty(N, np.int64)
    for k in range(N_CORES):
        b = bounds[k]
        s = sidx[k * nloc : (k + 1) * nloc]
        idx = np.arange(nloc)
        wk = np.searchsorted(b, idx, side="right") - 1
        win[k * nloc : (k + 1) * nloc] = wk
        rank[k * nloc : (k + 1) * nloc] = idx - b[wk]
        wbase[k, : len(b) - 1] = s[b[:-1]]

    k_arr = np.repeat(np.arange(N_CORES), nloc)
    lid_val = sidx - wbase[k_arr, win]
    # slot layout: [core][window][partition][tile] so each partition's DRAM
    # run within a window is contiguous
    slot = (k_arr * W + win) * cap + (rank & (P - 1)) * T + (rank >> 7)

    total = N_CORES * W * cap
    src = np.zeros(total, np.int64)
    src[slot] = perm

    hi = H.astype(NP_BF16)
    Hp = np.empty((total, planes, D), NP_BF16)
    Hp[:, 0, :] = hi[src]
    if planes == 2:
        lo = (H - hi.astype(np.float32)).astype(NP_BF16)
        Hp[:, 1, :] = lo[src]
    Hp = Hp.reshape(N_CORES, W, P, T * planes * D)

    lid = np.full(total, PAD_LID, NP_BF16)
    lid[slot] = lid_val.astype(NP_BF16)
    lid = (
        lid.reshape(N_CORES, W, P, T).transpose(0, 2, 1, 3).reshape(N_CORES, P, W * T)
    )
    lid = np.ascontiguousarray(lid)

    iota = np.ascontiguousarray(
        np.broadcast_to(np.arange(P, dtype=np.float32).astype(NP_BF16), (P, P))
    )

    in_maps = [{"h": Hp[k], "lid": lid[k], "iota": iota} for k in range(N_CORES)]
    return in_maps, wbase, W, T


def kernel(H, X_node, V, trace: bool = False) -> np.ndarray:
    global LAST_RESULTS
    H = np.asarray(H, dtype=np.float32)
    X = np.asarray(X_node)
    V = int(V)

    planes = int(os.environ.get("SEGSUM_PLANES", "2"))
    in_maps, wbase, W, T = _prepare(H, X, V, planes)
    nc = _build_nc_cached(W, T, planes)
    res = run_bass_kernel_spmd(nc, in_maps, list(range(N_CORES)), trace=trace)
    LAST_RESULTS = res

    out = np.zeros((V + P, D), np.float32)
    for k in range(N_CORES):
        o = np.asarray(res.results[k]["out"])
        for w in range(W):
            b = int(wbase[k, w])
            out[b : b + P] += o[w]
    return np.ascontiguousarray(out[:V])



# revision 4
# speedup vs baseline: 1.2191x; 1.2191x over previous
"""Trainium2 (Bass/Tile) segment-sum kernel, 8-core SPMD — v3 (lid-groups).

Computes out[v, :] = sum over rows n with X_node[n] == v of H[n, :]
(= jax.ops.segment_sum(H, X_node, num_segments=V)).

v3 structure (changes vs v2 in *bold*):
  host: stable-argsort rows by segment; 8 contiguous core chunks; greedy
    windows of <=128-segment span. *Within a window, each segment's rows
    are padded to a multiple of G and packed into "groups": G consecutive
    tile-columns at one partition that all hold rows of the SAME segment.*
    (~1.6% zero-row padding for G=2, ~4.7% for G=4.)
  device, per (window w, group q):
    - ONE DVE tensor_scalar builds the staircase u[p, v] = (iota[v] >=
      lid[p, q]) for ALL G columns of the group at once (they share lid).
    - ONE matmul with moving operand [128, G*128] (the group's G row-
      tiles, contiguous in SBUF) and a *stride-0 PSUM out AP* accumulates
      all G tiles into acc[v, :]: PSUM's has_written bit makes within-
      instruction revisits of the same element accumulate. One hidden
      LDWEIGHTS per G tiles keeps TensorE dense (HAM stays warm).
    PSUM strips hold cumulative-along-v segment sums as in v2.
  host: seg sums = diff of strips along v (prepend 0); add into out.

vs v2 measured (215.6us): DVE builds 1189x163ns=193us -> /G; PE
1566 MM + 1566 LDW (177+131us) -> T/G wide MMs, loads hidden. New
bound: DMA ~150-160us (51.2MB bf16 H * (1+pad) + 3.5MB f32 strips).
"""

import os

import numpy as np
from contextlib import ExitStack

import ml_dtypes
import concourse.bass as bass
import concourse.tile as tile
from concourse import bacc, mybir
from concourse.bass_utils import run_bass_kernel_spmd

F32 = mybir.dt.float32
BF16 = mybir.dt.bfloat16
NP_BF16 = ml_dtypes.bfloat16
P = 128  # partitions / tile rows / max window width (segments)
D = 128  # feature dim
N_CORES = 8
PAD_LID = 255.0

LAST_RESULTS = None  # test-harness hook: BassKernelResults of the last run
_NC_CACHE = {}  # (W, T, G) -> compiled Bacc program


def _t_candidates(G: int):
    # T must be a multiple of G (whole groups) and the half-split point
    # G*ceil(T/(2G)) stays G-aligned automatically.
    return tuple(t for t in (24, 26, 28, 30, 32) if t % G == 0)


def _build_nc_cached(W: int, T: int, G: int):
    key = (W, T, G)
    if key not in _NC_CACHE:
        _NC_CACHE[key] = _build_nc(W, T, G)
    return _NC_CACHE[key]


def _build_nc(W: int, T: int, G: int):
    Q = T // G  # groups per window
    nc = bacc.Bacc(
        "TRN2",
        target_bir_lowering=False,
        debug=False,
        enable_asserts=False,
        num_devices=N_CORES,
    )
    # h[w, p, (t, d)] -- per-partition contiguous runs of T*D*2B
    h = nc.dram_tensor("h", [W, P, T * D], BF16, kind="ExternalInput")
    lid = nc.dram_tensor("lid", [P, W * Q], BF16, kind="ExternalInput")
    iota = nc.dram_tensor("iota", [P, P], BF16, kind="ExternalInput")
    bdiag = nc.dram_tensor("bdiag", [P, P], F32, kind="ExternalInput")
    out = nc.dram_tensor("out", [W, P, D], BF16, kind="ExternalOutput")

    with tile.TileContext(nc) as tc, ExitStack() as ctx:
        const = ctx.enter_context(tc.tile_pool(name="const", bufs=1))
        hpool = ctx.enter_context(tc.tile_pool(name="hw", bufs=40))
        ohpool = ctx.enter_context(tc.tile_pool(name="oh", bufs=20))
        mspool = ctx.enter_context(tc.tile_pool(name="ms", bufs=8))
        opool = ctx.enter_context(tc.tile_pool(name="ot", bufs=16))
        psum = ctx.enter_context(tc.tile_pool(name="acc", bufs=6, space="PSUM"))
        psum2 = ctx.enter_context(tc.tile_pool(name="acc2", bufs=2, space="PSUM"))

        qh = (Q + 1) // 2  # groups in the first half-load
        halves = [(0, qh), (qh, Q)]  # in group units

        def load_h(w, q0, q1):
            ht = hpool.tile([P, (q1 - q0) * G * D], BF16, tag="ht")
            nc.sync.dma_start(ht[:], h[w][:, q0 * G * D : q1 * G * D])
            return ht

        hts = {}
        for w in range(min(3, W)):
            for q0, q1 in halves:
                hts[(w, q0)] = load_h(w, q0, q1)

        # constants go via the ACT ring so they land immediately instead of
        # queueing behind the hoisted bulk loads on the SP ring
        iota_sb = const.tile([P, P], BF16)
        nc.scalar.dma_start(iota_sb[:], iota[:])
        # lid ships as bf16 (exact for <=255) and is cast once to the f32
        # scratch that tensor_scalar's comparison scalar requires
        lid_bf = const.tile([P, W * Q], BF16)
        nc.scalar.dma_start(lid_bf[:], lid[:])
        lid_sb = const.tile([P, W * Q], F32)
        nc.scalar.copy(lid_sb[:], lid_bf[:])
        bdiag_sb = const.tile([P, P], F32)
        nc.scalar.dma_start(bdiag_sb[:], bdiag[:])

        for w in range(W):
            acc = psum.tile([P, D], F32)
            for q0, q1 in halves:
                if (w, q0) in hts:
                    ht = hts.pop((w, q0))
                else:
                    ht = load_h(w, q0, q1)
                for q in range(q0, q1):
                    # u[p, v] = (iota[v] >= lid[p, w*Q + q]); shared by the
                    # group's G tile-columns
                    oh = ohpool.tile([P, P], BF16)
                    nc.vector.tensor_scalar(
                        oh[:],
                        iota_sb[:],
                        lid_sb[:, w * Q + q : w * Q + q + 1],
                        None,
                        mybir.AluOpType.is_ge,
                    )
                    # one matmul streams the group's G tiles; the stride-0
                    # out AP revisits acc[v, :] G times -> PSUM accumulates
                    nc.tensor.matmul(
                        acc[:].unsqueeze(1).broadcast_to((P, G, D)),
                        oh[:],
                        ht[:, (q - q0) * G * D : (q - q0 + 1) * G * D],
                        start=(q == 0),
                        stop=(q == Q - 1),
                    )
            # diff along v on-device: one fp32 matmul with the fixed
            # bidiagonal B (B[p,p]=1, B[p,p+1]=-1) turns the cumulative
            # strip into per-segment sums exactly (+-1 multiplies), so the
            # now-small values ship as bf16 (halves the output DMA)
            ms = mspool.tile([P, D], F32)
            nc.scalar.copy(ms[:], acc[:])
            acc2 = psum2.tile([P, D], F32)
            nc.tensor.matmul(acc2[:], bdiag_sb[:], ms[:], start=True, stop=True)
            ot = opool.tile([P, D], BF16)
            nc.scalar.copy(ot[:], acc2[:])
            nc.scalar.dma_start(out[w], ot[:])

    nc.compile()
    return nc


def _prepare(H: np.ndarray, X: np.ndarray, V: int, G: int):
    """Host-side sort + greedy grouping + bf16 cast + swizzle.

    Returns (in_maps, wbase[k, w], W, T).
    """
    N, Dd = H.shape
    assert Dd == D and N % N_CORES == 0
    nloc = N // N_CORES
    X = np.ascontiguousarray(X).astype(np.int64, copy=False)
    perm = np.argsort(X, kind="stable")
    sidx = X[perm]

    # Per-core segment runs (pieces) in sorted order.
    def runs_for_core(k):
        s = sidx[k * nloc : (k + 1) * nloc]
        segs, counts = np.unique(s, return_counts=True)
        return segs, counts

    core_runs = [runs_for_core(k) for k in range(N_CORES)]

    def greedy(T):
        """Pack segments (splitting across windows allowed) into windows of
        <=128-seg span and <=128*(T/G) groups. Returns per-core window list:
        (wbase, [(seg, take_rows), ...]) and W."""
        Q = T // G
        capq = P * Q  # groups per window
        allw = []
        for k in range(N_CORES):
            segs, counts = core_runs[k]
            wins = []
            i, rem = 0, 0  # segment index, rows already consumed of segs[i]
            while i < len(segs):
                base = segs[i]
                used = 0
                pieces = []
                while i < len(segs) and segs[i] < base + P:
                    r = counts[i] - rem
                    g = (r + G - 1) // G
                    if used + g <= capq:
                        pieces.append((segs[i], r))
                        used += g
                        rem = 0
                        i += 1
                    else:
                        take = (capq - used) * G
                        if take > 0:
                            pieces.append((segs[i], take))
                            rem += take
                        used = capq
                        break
                wins.append((base, pieces))
            allw.append(wins)
        W = max(len(w) for w in allw)
        return allw, W

    best = None
    for T in _t_candidates(G):
        allw, W = greedy(T)
        if best is None or W * T < best[2] * best[1]:
            best = (allw, T, W)
    allw, T, W = best
    Q = T // G
    cap = T * P

    # Build the swizzled H, the per-group lid matrix, and wbase.
    total = N_CORES * W * cap
    hi = H.astype(NP_BF16)
    Hp = np.zeros((total, D), NP_BF16)
    lid_q = np.full((N_CORES, W, P, Q), PAD_LID, np.float32)
    wbase = np.full((N_CORES, W), V, np.int64)

    for k in range(N_CORES):
        row = k * nloc  # next sorted-row index to place
        for w, (base, pieces) in enumerate(allw[k]):
            wbase[k, w] = base
            j = 0  # group index within window
            for seg, r in pieces:
                g = (r + G - 1) // G
                jj = j + np.arange(g)
                lid_q[k, w, jj % P, jj // P] = seg - base
                # rows of this piece -> slots
                idx = np.arange(r)
                gj = j + idx // G  # global group idx
                p_ = gj % P
                t_ = (gj // P) * G + (idx % G)
                slot = ((k * W + w) * P + p_) * T + t_
                Hp[slot] = hi[perm[row : row + r]]
                row += r
                j += g
        assert row == (k + 1) * nloc

    Hp = Hp.reshape(N_CORES, W, P, T * D)
    lid_mat = np.ascontiguousarray(
        lid_q.transpose(0, 2, 1, 3).reshape(N_CORES, P, W * Q).astype(NP_BF16)
    )

    iota = np.ascontiguousarray(
        np.broadcast_to(np.arange(P, dtype=np.float32).astype(NP_BF16), (P, P))
    )
    bdiag = np.ascontiguousarray(
        np.eye(P, dtype=np.float32) - np.eye(P, k=1, dtype=np.float32)
    )

    in_maps = [
        {"h": Hp[k], "lid": lid_mat[k], "iota": iota, "bdiag": bdiag}
        for k in range(N_CORES)
    ]
    return in_maps, wbase, W, T


def kernel(H, X_node, V, trace: bool = False) -> np.ndarray:
    global LAST_RESULTS
    H = np.asarray(H, dtype=np.float32)
    X = np.asarray(X_node)
    V = int(V)

    G = int(os.environ.get("SEGSUM_G", "2"))
    in_maps, wbase, W, T = _prepare(H, X, V, G)
    nc = _build_nc_cached(W, T, G)
    res = run_bass_kernel_spmd(nc, in_maps, list(range(N_CORES)), trace=trace)
    LAST_RESULTS = res

    out = np.zeros((V + P, D), np.float32)
    for k in range(N_CORES):
        # strips are already diffed on-device: per-segment sums in bf16
        d = np.asarray(res.results[k]["out"]).astype(np.float32)
        for w in range(W):
            b = int(wbase[k, w])
            out[b : b + P] += d[w]
    return np.ascontiguousarray(out[:V])
